# revision 1
# baseline (speedup 1.0000x reference)
"""AttentionBlock (GroupNorm + 8-head attention + proj + residual) on 8 TRN2 cores.

Sharding: data-parallel over batch B=8 -> one image per NeuronCore, weights
replicated, no collectives. Host does layout prep (transposes / head
rearrangement / bf16 casts) and final reassembly.
"""
import sys
import types

import numpy as np
import ml_dtypes

import concourse.bass as bass
import concourse.tile as tile
from concourse import bacc, mybir
from concourse.bass_utils import run_bass_kernel_spmd

F32 = mybir.dt.float32
BF16 = mybir.dt.bfloat16

B, C, N = 8, 512, 1024          # batch, channels, H*W
NH, HD = 8, 64                  # heads, head_dim
G, GS = 32, 16                  # groups, channels per group
EPS = 1e-5
NCORES = 8
CT = C // 128                   # 4 channel tiles
ST = N // 128                   # 8 s-tiles
NCH = 2                         # t-chunks of 512
TRACE = False                   # set by test harness for profiling
DEBUG = False                   # extra debug outputs

_CACHE = {}


def _install_ntff_hook():
    """antenv.axon_hooks is absent in this container; inject it so
    run_bass_kernel_spmd(trace=True) can capture NTFF profiles."""
    if "antenv.axon_hooks" in sys.modules:
        return
    try:
        from trn_agent_boot.trn_boot import _ntff_profile_via_ctypes
        hook = _ntff_profile_via_ctypes("/opt/axon/libaxon_pjrt.so")
    except Exception:
        hook = None
    mod = types.ModuleType("antenv.axon_hooks")
    mod.get_axon_ntff_profile_hook = lambda: hook
    mod.set_axon_ntff_profile_hook = lambda h: None
    sys.modules["antenv.axon_hooks"] = mod


def build_nc(debug=False):
    nc = bacc.Bacc("TRN2", target_bir_lowering=False, debug=False,
                   num_devices=NCORES)
    x = nc.dram_tensor("x", (C, N), BF16, kind="ExternalInput").ap()
    xpb = nc.dram_tensor("xpb", (C, N), F32, kind="ExternalInput").ap()
    qkvw = nc.dram_tensor("qkvw", (C, 3 * C), BF16, kind="ExternalInput").ap()
    pw = nc.dram_tensor("pw", (C, C), BF16, kind="ExternalInput").ap()
    gnw = nc.dram_tensor("gnw", (128, CT), F32, kind="ExternalInput").ap()
    gnb = nc.dram_tensor("gnb", (128, CT), F32, kind="ExternalInput").ap()
    mask = nc.dram_tensor("mask", (128, 128), F32, kind="ExternalInput").ap()
    out = nc.dram_tensor("out", (C, N), F32, kind="ExternalOutput").ap()
    rs_scr = nc.dram_tensor("rs_scr", (NH, N), F32).ap()  # internal scratch

    dbg = {}
    if debug:
        for name, shape in [("d_xn", (C, N)), ("d_q", (C, N)), ("d_k", (C, N)),
                            ("d_vt", (N, NH * 65)), ("d_h", (C, N)),
                            ("d_rs", (NH, N))]:
            dbg[name] = nc.dram_tensor(name, shape, F32, kind="ExternalOutput").ap()

    x_t = x.rearrange("(t p) n -> p t n", p=128)
    xpb_t = xpb.rearrange("(t p) n -> p t n", p=128)
    qkvw_t = qkvw.rearrange("(t p) o -> p t o", p=128)
    pw_t = pw.rearrange("(t p) o -> p t o", p=128)
    out_t = out.rearrange("(t p) n -> p t n", p=128)

    with tile.TileContext(nc) as tc:
        with (
            tc.tile_pool(name="wpool", bufs=1) as wp,       # persistent
            tc.tile_pool(name="xres", bufs=1) as xres,      # x then xpb (slot reuse)
            tc.tile_pool(name="small", bufs=1) as sm,       # consts/stats
            tc.tile_pool(name="ppool", bufs=32) as pp,      # P = exp(S^T)
            tc.tile_pool(name="hrawp", bufs=8) as hrawp,    # h_raw tiles
            tc.tile_pool(name="rsbp", bufs=4) as rsbp,      # broadcast tiles
            tc.tile_pool(name="dbgp", bufs=2) as dbgp,      # debug dumps
            tc.tile_pool(name="rsp", bufs=2) as rsp,        # rowsum tiles
            tc.tile_pool(name="outp", bufs=2) as op_,       # output tiles
            tc.tile_pool(name="ps_mm", bufs=2, space="PSUM") as ps_mm,
            tc.tile_pool(name="ps_s", bufs=2, space="PSUM") as ps_s,
            tc.tile_pool(name="ps_h", bufs=2, space="PSUM") as ps_h,
        ):
            # ---- persistent SBUF ----
            qkvw_sb = wp.tile([128, CT, 3 * C], BF16, tag="qkvw")
            pw_sb = wp.tile([128, CT, C], BF16, tag="pw")
            xn_sb = wp.tile([128, CT, N], BF16, tag="xn")
            q_sb = wp.tile([128, CT, N], BF16, tag="q")
            k_sb = wp.tile([128, CT, N], BF16, tag="k")
            vt_sb = wp.tile([128, ST, NH, 65], BF16, tag="vt")
            h_sb = wp.tile([128, CT, N], BF16, tag="h")
            gnw_sb = wp.tile([128, CT], F32, tag="gnw")
            gnb_sb = wp.tile([128, CT], F32, tag="gnb")
            mask_sb = wp.tile([128, 128], F32, tag="mask")

            x_sb = wp.tile([128, CT, N], BF16, tag="xbf")

            # ---- input DMAs ----
            for ct in range(CT):
                nc.sync.dma_start(out=x_sb[:, ct, :], in_=x_t[:, ct, :])
            nc.sync.dma_start(out=qkvw_sb, in_=qkvw_t)
            nc.sync.dma_start(out=gnw_sb, in_=gnw)
            nc.sync.dma_start(out=gnb_sb, in_=gnb)
            nc.sync.dma_start(out=mask_sb, in_=mask)
            nc.vector.memset(vt_sb[:, :, :, 64:65], 1.0)

            eps_t = sm.tile([128, 1], F32, tag="eps")
            nc.vector.memset(eps_t, EPS)

            # ---- GroupNorm ----
            stats_in = sm.tile([128, 8], F32, tag="sin")
            for ct in range(CT):
                stats = sm.tile([128, 2, 6], F32, tag="bst")
                for j in range(2):
                    nc.vector.bn_stats(out=stats[:, j, :],
                                       in_=x_sb[:, ct, j * 512:(j + 1) * 512])
                mv = sm.tile([128, 2], F32, tag="mv")
                nc.vector.bn_aggr(out=mv, in_=stats)
                # cols: [mean x4 | E[x^2] x4]
                nc.vector.tensor_copy(stats_in[:, ct:ct + 1], mv[:, 0:1])
                msq = sm.tile([128, 1], F32, tag="msq")
                nc.vector.tensor_mul(msq, mv[:, 0:1], mv[:, 0:1])
                nc.vector.tensor_add(stats_in[:, 4 + ct:5 + ct], mv[:, 1:2], msq)
            stats_ps = ps_mm.tile([128, 8], F32, tag="mm")
            nc.tensor.matmul(stats_ps, mask_sb, stats_in, start=True, stop=True)
            stats_gs = sm.tile([128, 8], F32, tag="sgs")
            nc.vector.tensor_copy(stats_gs, stats_ps)
            means_g = stats_gs[:, 0:4]
            e2_g = stats_gs[:, 4:8]
            msq_g = sm.tile([128, 4], F32, tag="msqg")
            nc.vector.tensor_mul(msq_g, means_g, means_g)
            var_g = sm.tile([128, 4], F32, tag="varg")
            nc.vector.tensor_tensor(out=var_g, in0=e2_g, in1=msq_g,
                                    op=mybir.AluOpType.subtract)
            lnv = sm.tile([128, 4], F32, tag="lnv")
            nc.scalar.activation(out=lnv, in_=var_g,
                                 func=mybir.ActivationFunctionType.Ln,
                                 bias=eps_t, scale=1.0)
            rstd = sm.tile([128, 4], F32, tag="rstd")
            nc.scalar.activation(out=rstd, in_=lnv,
                                 func=mybir.ActivationFunctionType.Exp,
                                 bias=0.0, scale=-0.5)
            sc_g = sm.tile([128, 4], F32, tag="scg")
            nc.vector.tensor_mul(sc_g, rstd, gnw_sb)
            tmp_b = sm.tile([128, 4], F32, tag="tmpb")
            nc.vector.tensor_mul(tmp_b, means_g, sc_g)
            bias_g = sm.tile([128, 4], F32, tag="biag")
            nc.vector.tensor_tensor(out=bias_g, in0=gnb_sb, in1=tmp_b,
                                    op=mybir.AluOpType.subtract)
            for ct in range(CT):
                nc.vector.tensor_scalar(
                    out=xn_sb[:, ct, :], in0=x_sb[:, ct, :],
                    scalar1=sc_g[:, ct:ct + 1], scalar2=bias_g[:, ct:ct + 1],
                    op0=mybir.AluOpType.mult, op1=mybir.AluOpType.add)
            if debug:
                xn_f = dbgp.tile([128, N], F32, tag="dbgf")
                for ct in range(CT):
                    nc.vector.tensor_copy(xn_f, xn_sb[:, ct, :])
                    nc.sync.dma_start(out=dbg["d_xn"].rearrange(
                        "(t p) n -> p t n", p=128)[:, ct, :], in_=xn_f)

            # late: xpb for residual + proj weights
            xpb_sb = xres.tile([128, CT, N], F32, tag="xres")
            for ct in range(CT):
                nc.sync.dma_start(out=xpb_sb[:, ct, :], in_=xpb_t[:, ct, :])
            nc.sync.dma_start(out=pw_sb, in_=pw_t)

            P = {}       # P[head][stile] -> AP (128, N) bf16
            hraw = {}    # hraw[(head, chunk)] -> AP (65, 512) bf16

            def qk_mms(pair):
                for dst, base in ((q_sb, 0), (k_sb, C)):
                    for ch in range(NCH):
                        pt = ps_mm.tile([128, 512], F32, tag="mm")
                        for kt in range(CT):
                            nc.tensor.matmul(
                                pt,
                                qkvw_sb[:, kt, base + pair * 128:base + (pair + 1) * 128],
                                xn_sb[:, kt, ch * 512:(ch + 1) * 512],
                                start=(kt == 0), stop=(kt == CT - 1))
                        nc.vector.tensor_copy(dst[:, pair, ch * 512:(ch + 1) * 512], pt)

            def vt_mms():
                for st in range(ST):
                    pt = ps_mm.tile([128, 512], F32, tag="mm")
                    for kt in range(CT):
                        nc.tensor.matmul(
                            pt,
                            xn_sb[:, kt, st * 128:(st + 1) * 128],
                            qkvw_sb[:, kt, 2 * C:3 * C],
                            start=(kt == 0), stop=(kt == CT - 1))
                    nc.vector.tensor_copy(
                        vt_sb[:, st, :, 0:64],
                        pt.rearrange("p (h c) -> p h c", h=NH))

            def s_exp(pair):
                for st in range(ST):
                    for h01 in range(2):
                        head = 2 * pair + h01
                        lo, hi = h01 * 64, (h01 + 1) * 64
                        spt = ps_s.tile([128, N], F32, tag="s")
                        for ch in range(NCH):
                            nc.tensor.matmul(
                                spt[:, ch * 512:(ch + 1) * 512],
                                k_sb[lo:hi, pair, st * 128:(st + 1) * 128],
                                q_sb[lo:hi, pair, ch * 512:(ch + 1) * 512],
                                start=True, stop=True)
                        ptile = pp.tile([128, N], BF16, tag="P")
                        nc.scalar.activation(
                            out=ptile, in_=spt,
                            func=mybir.ActivationFunctionType.Exp,
                            bias=0.0, scale=1.0)
                        P.setdefault(head, {})[st] = ptile

            def h_stage(pair):
                for h01 in range(2):
                    head = 2 * pair + h01
                    for ch in range(NCH):
                        hpt = ps_h.tile([65, 512], F32, tag="hps")
                        for st in range(ST):
                            nc.tensor.matmul(
                                hpt,
                                vt_sb[:, st, head, :],
                                P[head][st][:, ch * 512:(ch + 1) * 512],
                                start=(st == 0), stop=(st == ST - 1))
                        hr = hrawp.tile([65, 512], BF16, tag="hraw")
                        nc.vector.tensor_copy(hr, hpt)
                        hraw[(head, ch)] = hr
                # rowsum reciprocal for the pair
                rs_bf = rsp.tile([2, N], BF16, tag="rsbf")
                for h01 in range(2):
                    for ch in range(NCH):
                        nc.sync.dma_start(
                            out=rs_bf[h01:h01 + 1, ch * 512:(ch + 1) * 512],
                            in_=hraw[(2 * pair + h01, ch)][64:65, :])
                rs_f = rsp.tile([2, N], F32, tag="rsf")
                nc.vector.tensor_copy(rs_f, rs_bf)
                rsr = rsp.tile([2, N], F32, tag="rsr")
                nc.vector.reciprocal_approx_fast(out=rsr, in_=rs_f)
                nc.sync.dma_start(out=rs_scr[2 * pair:2 * pair + 2, :], in_=rsr)
                # broadcast back + normalize into h_sb
                for h01 in range(2):
                    head = 2 * pair + h01
                    for ch in range(NCH):
                        rsb = rsbp.tile([64, 512], BF16, tag="rsb")
                        nc.gpsimd.dma_start(
                            out=rsb,
                            in_=rs_scr[head:head + 1, ch * 512:(ch + 1) * 512]
                            .to_broadcast([64, 512]))
                        nc.vector.tensor_tensor(
                            out=h_sb[h01 * 64:(h01 + 1) * 64, pair,
                                     ch * 512:(ch + 1) * 512],
                            in0=hraw[(head, ch)][0:64, :], in1=rsb,
                            op=mybir.AluOpType.mult)

            # ---- pipeline ----
            qk_mms(0)
            s_exp(0)
            vt_mms()
            for pair in range(1, 4):
                qk_mms(pair)
                h_stage(pair - 1)
                s_exp(pair)
            h_stage(3)

            if debug:
                for name, src in [("d_q", q_sb), ("d_k", k_sb), ("d_h", h_sb)]:
                    for ct in range(CT):
                        f = dbgp.tile([128, N], F32, tag="dbgf")
                        nc.vector.tensor_copy(f, src[:, ct, :])
                        nc.sync.dma_start(out=dbg[name].rearrange(
                            "(t p) n -> p t n", p=128)[:, ct, :], in_=f)
                for st in range(ST):
                    f = dbgp.tile([128, NH * 65], F32, tag="dbgv")
                    nc.vector.tensor_copy(
                        f.rearrange("p (h c) -> p h c", h=NH), vt_sb[:, st, :, :])
                    nc.sync.dma_start(out=dbg["d_vt"].rearrange(
                        "(t p) c -> p t c", p=128)[:, st, :], in_=f)
                f = dbgp.tile([NH, N], F32, tag="dbgr")
                nc.sync.dma_start(out=f, in_=rs_scr)
                nc.sync.dma_start(out=dbg["d_rs"], in_=f)

            # ---- proj + residual ----
            for ot in range(CT):
                osb = op_.tile([128, N], F32, tag="osb")
                for ch in range(NCH):
                    if (ot * NCH + ch) % 2 == 0:
                        pt = ps_mm.tile([128, 512], F32, tag="mm")
                    else:
                        pt = ps_s.tile([128, 512], F32, tag="s")
                    for kt in range(CT):
                        nc.tensor.matmul(
                            pt,
                            pw_sb[:, kt, ot * 128:(ot + 1) * 128],
                            h_sb[:, kt, ch * 512:(ch + 1) * 512],
                            start=(kt == 0), stop=(kt == CT - 1))
                    nc.vector.tensor_tensor(
                        out=osb[:, ch * 512:(ch + 1) * 512], in0=pt,
                        in1=xpb_sb[:, ot, ch * 512:(ch + 1) * 512],
                        op=mybir.AluOpType.add)
                nc.sync.dma_start(out=out_t[:, ot, :], in_=osb)

    nc.finalize()
    return nc


def make_in_maps(x, gn_w, gn_b, qkv_w, proj_w, proj_b):
    x = np.asarray(x, dtype=np.float32).reshape(B, C, N)
    gn_w = np.asarray(gn_w, dtype=np.float32)
    gn_b = np.asarray(gn_b, dtype=np.float32)
    qkv_w = np.asarray(qkv_w, dtype=np.float32)
    proj_w = np.asarray(proj_w, dtype=np.float32)
    proj_b = np.asarray(proj_b, dtype=np.float32)

    scale = 1.0 / np.sqrt(np.sqrt(HD))
    # reference splits qkv rows per head: head h rows [h*192, h*192+192) = q|k|v
    rows = qkv_w.reshape(NH, 3, HD, C)
    qw = rows[:, 0].reshape(C, C) * scale
    kw = rows[:, 1].reshape(C, C) * scale
    vw = rows[:, 2].reshape(C, C)
    qkvw_t = np.ascontiguousarray(
        np.concatenate([qw, kw, vw], axis=0).T).astype(ml_dtypes.bfloat16)
    pw_t = np.ascontiguousarray(proj_w.T).astype(ml_dtypes.bfloat16)
    gnw_dev = np.ascontiguousarray(gn_w.reshape(CT, 128).T)
    gnb_dev = np.ascontiguousarray(gn_b.reshape(CT, 128).T)
    mask = np.zeros((128, 128), dtype=np.float32)
    for g in range(8):
        mask[g * GS:(g + 1) * GS, g * GS:(g + 1) * GS] = 1.0 / GS

    in_maps = []
    for b in range(B):
        xc = np.ascontiguousarray(x[b])
        in_maps.append({
            "x": xc.astype(ml_dtypes.bfloat16),
            "xpb": np.ascontiguousarray(xc + proj_b[:, None]),
            "qkvw": qkvw_t, "pw": pw_t,
            "gnw": gnw_dev, "gnb": gnb_dev, "mask": mask,
        })
    return in_maps


def kernel(x, gn_w, gn_b, qkv_w, proj_w, proj_b, num_heads):
    assert int(num_heads) == NH
    _install_ntff_hook()
    in_maps = make_in_maps(x, gn_w, gn_b, qkv_w, proj_w, proj_b)
    if "nc" not in _CACHE:
        _CACHE["nc"] = build_nc(debug=DEBUG)
    r = run_bass_kernel_spmd(_CACHE["nc"], in_maps,
                             core_ids=list(range(NCORES)), trace=TRACE)
    _CACHE["last_result"] = r
    out = np.stack([np.asarray(r.results[b]["out"], dtype=np.float32)
                    for b in range(B)])
    return out.reshape(B, C, 32, 32)

